# revision 19
# baseline (speedup 1.0000x reference)
"""Trainium2 Bass kernel for a pre-LN transformer block (B=2, T=2048, C=1024, H=16).

Strategy (8 NeuronCores, SPMD), v3 pipelined:
  - Tensor-parallel over heads for attention: core c owns heads {2c, 2c+1}.
  - LN1 stats distributed: core c computes mean/rsqrt rows for token chunk c
    only, then one 4KB AllGather shares all 8 chunks' stats; the LN1 affine is
    folded post-matmul into qkv as before.
  - jj-major software pipeline: for each q-chunk jj (512 tokens per batch),
    emit qkv for chunks (b0,jj),(b1,jj), then causal attention for q-chunk jj
    (all keys <= jj are ready), then a small per-jj AllToAll. Keeps the PE
    dense (HAM stays warm) and overlaps attention's Scalar-bound exp with the
    next chunk's matmuls.
  - Row ownership is 64-token interleaved: core c owns rows [64c, 64c+64) of
    every (batch, jj) 512-chunk, so each per-jj A2A feeds every core and only
    the last small A2A is exposed.
  - exp batched 2 key-tiles per ACT ([128,1024] PSUM read) to amortize the
    352-cycle ACT overhead; causal masks multiplied on VectorE (bf16 2x).
  - Row-parallel proj + LN2 + MLP on the core's own 512 rows.

PSUM budget (8 banks): tag "w2" = 3 x [128,1024] f32 (6 banks) +
tag "po" = 2 x [65,512] f32 (2 banks).
"""

import os
import numpy as np
import ml_dtypes

from concourse import bass, bacc, tile, mybir, bass_utils

BF16 = mybir.dt.bfloat16
F32 = mybir.dt.float32
F32R = mybir.dt.float32r
AX = mybir.AxisListType
OP = mybir.AluOpType
AF = mybir.ActivationFunctionType

B, T, C, H, HD = 2, 2048, 1024, 16, 64
NCORES = 8
BT = B * T                  # 4096 global tokens
RPC = BT // NCORES          # 512 rows per core
NCH = BT // 512             # 8 token chunks of 512
CB = C // 128               # 8 contraction blocks
HT = 4 * C // 128           # 32 hidden tiles
EPS = 1e-5

_cache = {}


def _fr(ap):
    return ap.bitcast(F32R)


def build():
    nc = bacc.Bacc("TRN2", target_bir_lowering=False, debug=False, num_devices=NCORES)

    def din(name, shape, dt=BF16):
        return nc.dram_tensor(name, list(shape), dt, kind="ExternalInput").ap()

    xT8 = din("xT8", [NCH, 128, 8 * 512])                 # x transposed, chunked
    xown = din("xown", [128, 8 * 512])                    # own chunk (stats)
    xrows = din("xrows", [4, 128, C], F32)                # own residual rows
    wqk = din("wqk", [CB, 128, 256])                      # [q_h0|q_h1|k_h0|k_h1]
    bqk = din("bqk", [2, 256, 1], F32)
    cqk = din("cqk", [2, 256, 1], F32)   # [b0-order, b1-swapped]
    wv = din("wv", [CB, 128, 128])                        # [v_h0|v_h1]
    bv = din("bv", [128, 1], F32)
    cv = din("cv", [128, 1], F32)
    wproj = din("wproj", [4, 128, 4 * 512])
    bproj = din("bproj", [1, C], F32R)
    wfc = din("wfc", [8, 128, 8 * 512])                   # htg -> [cb | 4 ht cols]
    bfc = din("bfc", [128, HT], F32)
    wfcp = din("wfcp", [HT, 128, 1024])
    bfcp = din("bfcp", [1, C], F32R)
    maskd = din("maskd", [4, 128, 512])                   # 0/1 causal diag masks
    ident = din("ident", [128, 128])
    onesr = din("onesr", [1, 128], F32R)
    out_rows = nc.dram_tensor("out_rows", [4, 128, C], F32, kind="ExternalOutput").ap()

    with tile.TileContext(nc) as tc:
        with tc.tile_pool(name="persist", bufs=1) as pp, \
             tc.tile_pool(name="work", bufs=2) as wk, \
             tc.tile_pool(name="psum", bufs=1, space="PSUM") as ps, \
             tc.tile_pool(name="dram", bufs=1, space="DRAM") as dram:

            # ---------- constants / small persistent tiles ----------
            ones_bf = pp.tile([128, 1], BF16, tag="ones_bf")
            nc.vector.memset(ones_bf[:], 1.0 / C)
            ones_row = pp.tile([1, 128], F32R, tag="ones_row")
            nc.sync.dma_start(ones_row[:], onesr[:])
            eps1 = pp.tile([1, 1], F32, tag="eps1")
            nc.vector.memset(eps1[:], EPS)
            eps128 = pp.tile([128, 1], F32, tag="eps128")
            nc.vector.memset(eps128[:], EPS)
            idn = pp.tile([128, 128], BF16, tag="idn")
            nc.sync.dma_start(idn[:], ident[:])
            msk = [pp.tile([128, 512], BF16, tag=f"msk{m}", name=f"msk{m}") for m in range(4)]
            for m in range(4):
                nc.sync.dma_start(msk[m][:], maskd[m])
            bq_sb = [pp.tile([128, 1], F32, tag=f"bq{v_}", name=f"bq{v_}") for v_ in range(2)]
            bk_sb = [pp.tile([128, 1], F32, tag=f"bk{v_}", name=f"bk{v_}") for v_ in range(2)]
            cq_sb = [pp.tile([128, 1], F32, tag=f"cq{v_}", name=f"cq{v_}") for v_ in range(2)]
            ck_sb = [pp.tile([128, 1], F32, tag=f"ck{v_}", name=f"ck{v_}") for v_ in range(2)]
            for v_ in range(2):
                nc.sync.dma_start(bq_sb[v_][:], bqk[v_, 0:128, :])
                nc.sync.dma_start(bk_sb[v_][:], bqk[v_, 128:256, :])
                nc.sync.dma_start(cq_sb[v_][:], cqk[v_, 0:128, :])
                nc.sync.dma_start(ck_sb[v_][:], cqk[v_, 128:256, :])
            bv_sb = pp.tile([128, 1], F32, tag="bv_sb")
            nc.sync.dma_start(bv_sb[:], bv[:])
            cv_sb = pp.tile([128, 1], F32, tag="cv_sb")
            nc.sync.dma_start(cv_sb[:], cv[:])
            bproj_sb = pp.tile([1, C], F32R, tag="bproj_sb")
            nc.sync.dma_start(bproj_sb[:], bproj[:])
            bfc_sb = pp.tile([128, HT], F32, tag="bfc_sb")
            nc.sync.dma_start(bfc_sb[:], bfc[:])
            bfcp_sb = pp.tile([1, C], F32R, tag="bfcp_sb")
            nc.sync.dma_start(bfcp_sb[:], bfcp[:])
            wqk_sb = [pp.tile([128, 256], BF16, tag=f"wqk{cb}", name=f"wqk{cb}") for cb in range(CB)]
            wv_sb = [pp.tile([128, 128], BF16, tag=f"wv{cb}", name=f"wvsb{cb}") for cb in range(CB)]
            for cb in range(CB):
                nc.sync.dma_start(wqk_sb[cb][:], wqk[cb])
                nc.sync.dma_start(wv_sb[cb][:], wv[cb])

            # persistent activation tensors, head-major:
            qt = [pp.tile([128, T], BF16, tag=f"qt{X}", name=f"qt{X}") for X in range(2)]
            kt_ = [pp.tile([128, T], BF16, tag=f"kt{X}", name=f"ktt{X}") for X in range(2)]
            # v transposed, rows = key position within its 128-tile:
            # v1big[b][:, 130*kti + 65*h + d], col 65*h+64 = ones (denominator)
            v1big = [pp.tile([128, 16 * 130], BF16, tag=f"v1b{b}", name=f"v1b{b}")
                     for b in range(B)]
            for b in range(B):
                vr = v1big[b].rearrange("p (k h d) -> p k h d", k=16, h=2)
                nc.vector.memset(vr[:, :, :, 64:65], 1.0)

            # residual rows + A2A-assembled yT
            x2 = [pp.tile([128, C], F32, tag=f"x2{tt}", name=f"x2{tt}") for tt in range(4)]
            for tt in range(4):
                nc.sync.dma_start(x2[tt][:], xrows[tt])
            yTbig = pp.tile([128, 8 * 512], BF16, tag="yTbig")
            wpj = [pp.tile([128, 4 * 512], BF16, tag=f"wpj{q}", name=f"wpj{q}")
                   for q in range(4)]
            for q in range(4):
                nc.sync.dma_start(wpj[q][:], wproj[q])
            ln2Tbig = pp.tile([128, 8 * 512], BF16, tag="ln2Tbig")
            ghT = [pp.tile([128, 512], BF16, tag=f"ghT{ht}", name=f"ghT{ht}") for ht in range(HT)]

            # collective DRAM buffers
            cci = dram.tile([1, 1024], F32, tag="cci")
            cco = dram.tile([8, 1024], F32, tag="cco")
            ib = [dram.tile([8, 128, 128], BF16, tag=f"ib{j}", name=f"ib{j}") for j in range(4)]
            ob = [dram.tile([8, 128, 128], BF16, tag=f"ob{j}", name=f"ob{j}") for j in range(4)]

            # ===== Stats pass: own chunk only, then AllGather =====
            sx = wk.tile([128, 8 * 512], BF16, tag="xt", bufs=2, name="sx")
            nc.sync.dma_start(sx[:], xown[:])
            st1 = ps.tile([1, 512], F32, tag="po", bufs=2)
            st2 = ps.tile([1, 512], F32, tag="po", bufs=2)
            for pt in range(CB):
                nc.tensor.matmul(st1[:], ones_bf[:], sx[:, 512 * pt:512 * (pt + 1)],
                                 start=(pt == 0), stop=(pt == CB - 1))
            for h in range(2):
                sqa = wk.tile([128, 4 * 512], BF16, tag="sq", bufs=1, name="sqa")
                nc.vector.tensor_tensor(
                    sqa[:], sx[:, 2048 * h:2048 * (h + 1)],
                    sx[:, 2048 * h:2048 * (h + 1)], op=OP.mult)
                for pp_i in range(4):
                    pt = 4 * h + pp_i
                    nc.tensor.matmul(st2[:], ones_bf[:],
                                     sqa[:, 512 * pp_i:512 * (pp_i + 1)],
                                     start=(pt == 0), stop=(pt == CB - 1))
            mu2 = wk.tile([1, 512], F32, tag="arow", bufs=2, name="mu2")
            nc.scalar.activation(mu2[:], st1[:], AF.Square)
            var = wk.tile([1, 512], F32, tag="arow", bufs=2, name="var")
            nc.vector.tensor_tensor(var[:], st2[:], mu2[:], op=OP.subtract)
            stat2 = wk.tile([1, 1024], F32, tag="stat2", bufs=1, name="stat2")
            nc.scalar.activation(stat2[0:1, 0:512], var[:], AF.Abs_reciprocal_sqrt, bias=eps1[:])
            nc.vector.tensor_tensor(stat2[0:1, 512:1024], st1[:], stat2[0:1, 0:512], op=OP.mult)
            nc.sync.dma_start(cci[:], stat2[:])
            nc.gpsimd.collective_compute(
                "AllGather", OP.bypass,
                ins=[cci.opt()], outs=[cco.opt()],
                replica_groups=[list(range(NCORES))],
            )


            # ---------- helpers ----------
            KMINI = int(os.environ.get("KMINI", "0"))
            def emit_chunk(jj, b):
                """LN1 (folded post-matmul) + qkv (transposed) for chunk (b, jj)."""
                ch = 4 * b + jj
                xta = wk.tile([128, 8 * 512], BF16, tag="xt", bufs=2, name="xta")
                nc.sync.dma_start(xta[:], xT8[ch])
                xt = [xta[:, 512 * pt:512 * (pt + 1)] for pt in range(CB)]
                strow = wk.tile([1, 1024], F32, tag="strow", bufs=2, name="strow")
                nc.scalar.dma_start(strow[:], cco[ch:ch + 1, :])
                bc_rs = wk.tile([128, 512], F32, tag="bc", bufs=2, name="bc_rs")
                bc_a = wk.tile([128, 512], F32, tag="bc", bufs=2, name="bc_a")
                nc.gpsimd.partition_broadcast(bc_rs[:], strow[0:1, 0:512])
                nc.gpsimd.partition_broadcast(bc_a[:], strow[0:1, 512:1024])

                def qk_mms(psum, base):
                    for pt in range(CB):
                        if b == 0:
                            nc.tensor.matmul(psum, wqk_sb[pt][:, base:base + 128],
                                             xt[pt], start=(pt == 0),
                                             stop=(pt == CB - 1))
                        else:
                            nc.tensor.matmul(psum[0:64, :],
                                             wqk_sb[pt][:, base + 64:base + 128],
                                             xt[pt], start=(pt == 0),
                                             stop=(pt == CB - 1))
                            nc.tensor.matmul(psum[64:128, :],
                                             wqk_sb[pt][:, base:base + 64],
                                             xt[pt], start=(pt == 0),
                                             stop=(pt == CB - 1),
                                             tile_position=(0, 64))

                def fold_qk(dst_pair, gp, cs_ap, b_ap):
                    # dst = rs*G - (bc_a*cs - b), written per partition half
                    m = wk.tile([128, 512], BF16, tag="foldm", bufs=2, name="m")
                    nc.vector.tensor_scalar(m[:], bc_a[:], cs_ap, b_ap,
                                            op0=OP.mult, op1=OP.subtract)
                    p1 = wk.tile([128, 512], BF16, tag="foldp", bufs=2, name="p1")
                    nc.vector.tensor_tensor(p1[:], gp, bc_rs[:], op=OP.mult)
                    js = slice(512 * jj, 512 * (jj + 1))
                    nc.vector.tensor_tensor(dst_pair[0][0:64, js], p1[0:64, :],
                                            m[0:64, :], op=OP.subtract)
                    nc.vector.tensor_tensor(dst_pair[1][64:128, js], p1[64:128, :],
                                            m[64:128, :], op=OP.subtract)

                dq = (qt[0], qt[1]) if b == 0 else (qt[1], qt[0])
                dk = (kt_[0], kt_[1]) if b == 0 else (kt_[1], kt_[0])
                qk2 = ps.tile([128, 1024], F32, tag="w2", bufs=3)
                qk_mms(qk2[:, 0:512], 0)
                qk_mms(qk2[:, 512:1024], 128)
                fold_qk(dq, qk2[:, 0:512], cq_sb[b][:], bq_sb[b][:])
                fold_qk(dk, qk2[:, 512:1024], ck_sb[b][:], bk_sb[b][:])
                # v (transposed) then per-128 transpose into row-layout v1big
                pv2 = ps.tile([128, 1024], F32, tag="w2", bufs=3)
                pv = pv2[:, 0:512]
                for pt in range(CB):
                    nc.tensor.matmul(pv, wv_sb[pt][:], xt[pt],
                                     start=(pt == 0), stop=(pt == CB - 1))
                vts = wk.tile([128, 512], BF16, tag="vts", bufs=2)
                m = wk.tile([128, 512], BF16, tag="foldm", bufs=2, name="m")
                nc.vector.tensor_scalar(m[:], bc_a[:], cv_sb[:], bv_sb[:],
                                        op0=OP.mult, op1=OP.subtract)
                p1 = wk.tile([128, 512], BF16, tag="foldp", bufs=2, name="p1")
                nc.vector.tensor_tensor(p1[:], pv, bc_rs[:], op=OP.mult)
                nc.vector.tensor_tensor(vts[:], p1[:], m[:], op=OP.subtract)
                ptr4 = ps.tile([128, 512], BF16, tag="po", bufs=2)
                for t in range(4):
                    nc.tensor.transpose(ptr4[:, 128 * t:128 * (t + 1)],
                                        vts[:, 128 * t:128 * (t + 1)], idn[:])
                dst = v1big[b].rearrange("p (k h d) -> p k h d", k=16, h=2)
                nc.vector.tensor_copy(
                    dst[:, 4 * jj:4 * jj + 4, :, 0:64],
                    ptr4[:].rearrange("p (k h d) -> p k h d", k=4, h=2))

            def emit_attn(jj):
                """Causal attention for q-chunk jj (both heads X, both batches)."""
                nkt = 4 * jj + 4
                for X in range(2):
                    for u in range(2):
                        b = u if X == 0 else 1 - u
                        po_t = ps.tile([65, 512], F32, tag="po", bufs=2)
                        for mb in range(nkt // 2):
                            s2 = ps.tile([128, 1024], F32, tag="w2", bufs=3)
                            for i in range(2):
                                kti = 2 * mb + i
                                nc.tensor.matmul(
                                    s2[:, 512 * i:512 * (i + 1)],
                                    kt_[X][64 * u:64 * (u + 1), 128 * kti:128 * (kti + 1)],
                                    qt[X][64 * u:64 * (u + 1), 512 * jj:512 * (jj + 1)],
                                    start=True, stop=True)
                            pt_sb = wk.tile([128, 1024], BF16, tag="ptb", bufs=3, name="pt_sb")
                            nc.scalar.activation(pt_sb[:], s2[:], AF.Exp)
                            for i in range(2):
                                kti = 2 * mb + i
                                mrel = kti - 4 * jj
                                sl = pt_sb[:, 512 * i:512 * (i + 1)]
                                if mrel >= 0:
                                    nc.vector.tensor_tensor(sl, sl, msk[mrel][:], op=OP.mult)
                                nc.tensor.matmul(
                                    po_t[:],
                                    v1big[b][:, 130 * kti + 65 * X:130 * kti + 65 * X + 65],
                                    sl, start=(kti == 0), stop=(kti == nkt - 1))
                        dcp = wk.tile([1, 512], F32, tag="dcp", bufs=1, name="dcp")
                        nc.vector.tensor_copy(dcp[:], po_t[64:65, :])
                        recip = wk.tile([1, 512], F32, tag="recip", bufs=1)
                        nc.vector.reciprocal_approx_fast(recip[:], dcp[:])
                        bcp = wk.tile([64, 512], F32, tag="bcb", bufs=2, name="bcp")
                        nc.gpsimd.partition_broadcast(bcp[:], recip[:])
                        yt = wk.tile([64, 512], BF16, tag="yt", bufs=2)
                        nc.vector.tensor_tensor(yt[:], po_t[0:64, :], bcp[:], op=OP.mult)
                        nc.sync.dma_start(
                            ib[jj][:, 64 * X:64 * X + 64, 64 * b:64 * b + 64]
                            .rearrange("c p t -> p c t"),
                            yt[:].rearrange("p (c t) -> p c t", c=8))
                nc.gpsimd.collective_compute(
                    "AllToAll", OP.bypass,
                    ins=[ib[jj].opt()], outs=[ob[jj].opt()],
                    replica_groups=[list(range(NCORES))],
                )

            def emit_assembly(jj):
                """Scatter A2A output ob[jj] into yTbig columns."""
                dst = yTbig.rearrange("p (s b g t) -> p s b g t", s=8, b=2, g=4)
                for b in range(2):
                    nc.sync.dma_start(
                        dst[:, :, b, jj, :],
                        ob[jj][:, :, 64 * b:64 * b + 64].rearrange("s p t -> p s t"))

            def emit_proj(tt):
                """proj + residual for token tile tt (128 own rows)."""
                pps = ps.tile([128, 1024], F32, tag="w2", bufs=3)
                for nh in range(2):
                    dst = pps[:, 512 * nh:512 * (nh + 1)]
                    for cbh in range(2):
                        q = 2 * nh + cbh
                        for cbl in range(4):
                            cb = 4 * cbh + cbl
                            nc.tensor.matmul(
                                dst,
                                yTbig[:, 512 * cb + 128 * tt:512 * cb + 128 * (tt + 1)],
                                wpj[q][:, 512 * cbl:512 * (cbl + 1)],
                                start=(cb == 0), stop=False)
                    nc.tensor.matmul(dst, ones_row[0:1, :],
                                     bproj_sb[0:1, 512 * nh:512 * (nh + 1)],
                                     start=False, stop=True)
                nc.vector.tensor_tensor(x2[tt][:], pps[:], x2[tt][:], op=OP.add)

            def emit_ln2(tt):
                """LN2 + transpose into ln2Tbig for token tile tt."""
                s1 = wk.tile([128, 1], F32, tag="e_s1")
                nc.vector.reduce_sum(s1[:], x2[tt][:], axis=AX.X)
                nmu = wk.tile([128, 1], F32, tag="e_nmu")
                nc.vector.tensor_scalar(nmu[:], s1[:], -1.0 / C, None, op0=OP.mult)
                sqs = wk.tile([128, C], F32, tag="sq", bufs=1, name="sqs")
                s2_ = wk.tile([128, 1], F32, tag="e_s2")
                nc.scalar.activation(sqs[:], x2[tt][:], AF.Square, accum_out=s2_[:])
                m2 = wk.tile([128, 1], F32, tag="e_m2")
                nc.vector.tensor_tensor(m2[:], nmu[:], nmu[:], op=OP.mult)
                var_ = wk.tile([128, 1], F32, tag="e_var")
                nc.vector.tensor_scalar(var_[:], s2_[:], 1.0 / C, None, op0=OP.mult)
                nc.vector.tensor_tensor(var_[:], var_[:], m2[:], op=OP.subtract)
                sd = wk.tile([128, 1], F32, tag="e_sd")
                nc.scalar.activation(sd[:], var_[:], AF.Sqrt, bias=eps128[:])
                rs2 = wk.tile([128, 1], F32, tag="e_rs2")
                nc.vector.reciprocal(rs2[:], sd[:])
                na = wk.tile([128, 1], F32, tag="e_na")
                nc.vector.tensor_tensor(na[:], nmu[:], rs2[:], op=OP.mult)
                lr = wk.tile([128, C], BF16, tag="e_lr", bufs=1)
                nc.scalar.activation(lr[:], x2[tt][:], AF.Identity,
                                     bias=na[:], scale=rs2[:])
                ldst = ln2Tbig.rearrange("p (cb t) -> p cb t", cb=8)
                for hh in range(2):
                    ptr4 = ps.tile([128, 512], BF16, tag="po", bufs=2)
                    for t in range(4):
                        cb = 4 * hh + t
                        nc.tensor.transpose(ptr4[:, 128 * t:128 * (t + 1)],
                                            lr[:, 128 * cb:128 * (cb + 1)], idn[:])
                    nc.vector.tensor_copy(
                        ldst[:, 4 * hh:4 * hh + 4, 128 * tt:128 * (tt + 1)],
                        ptr4[:].rearrange("p (k t) -> p k t", k=4))

            # =================== main pipeline ===================
            if KMINI:
                emit_chunk(0, 0)
                emit_chunk(0, 1)
                emit_attn(0)
                emit_assembly(0)
            else:
                for jj in range(4):
                    emit_chunk(jj, 0)
                    emit_chunk(jj, 1)
                    if jj == 3:
                        emit_proj(0)
                        emit_ln2(0)
                        emit_proj(2)
                        emit_ln2(2)
                    emit_attn(jj)
                    if jj >= 1:
                        emit_assembly(jj - 1)
                emit_assembly(3)
                emit_proj(1)
                emit_ln2(1)
                emit_proj(3)
                emit_ln2(3)

            # =========== MLP: fc + gelu ===========
            for htg in range(0 if KMINI else 8):
                wt_lo = wk.tile([128, 4 * 512], BF16, tag="wfc", bufs=2, name="wt_lo")
                nc.sync.dma_start(wt_lo[:], wfc[htg, :, 0:2048])
                wt_hi = wk.tile([128, 4 * 512], BF16, tag="wfc", bufs=2, name="wt_hi")
                nc.sync.dma_start(wt_hi[:], wfc[htg, :, 2048:4096])
                for sub in range(4):
                    ht = 4 * htg + sub
                    ph = ps.tile([128, 512], F32, tag="po", bufs=2)
                    for cb in range(CB):
                        wt = wt_lo if cb < 4 else wt_hi
                        cbl = cb % 4
                        nc.tensor.matmul(
                            ph[:], wt[:, 512 * cbl + 128 * sub:512 * cbl + 128 * (sub + 1)],
                            ln2Tbig[:, 512 * cb:512 * (cb + 1)],
                            start=(cb == 0), stop=(cb == CB - 1))
                    nc.scalar.activation(ghT[ht][:], ph[:], AF.Gelu,
                                         bias=bfc_sb[:, ht:ht + 1])

            # =========== MLP: fc_proj + residual -> output rows ===========
            pg = {}
            for tt in range(0 if KMINI else 3):
                pg[tt] = ps.tile([128, 1024], F32, tag="w2", bufs=3, name=f"pg{tt}")
            if not KMINI:
                pg3a = ps.tile([128, 512], F32, tag="po", bufs=2, name="pg3a")
                pg3b = ps.tile([128, 512], F32, tag="po", bufs=2, name="pg3b")

            def pg_dst(tt, nh):
                if tt < 3:
                    return pg[tt][:, 512 * nh:512 * (nh + 1)]
                return pg3a[:] if nh == 0 else pg3b[:]

            for ht in range(0 if KMINI else HT):
                w = wk.tile([128, 1024], BF16, tag="wfcp", bufs=3, name="wfcp")
                nc.gpsimd.dma_start(w[:], wfcp[ht])
                for tt in range(4):
                    for nh in range(2):
                        nc.tensor.matmul(pg_dst(tt, nh),
                                         ghT[ht][:, 128 * tt:128 * (tt + 1)],
                                         w[:, 512 * nh:512 * (nh + 1)],
                                         start=(ht == 0), stop=False)
            for tt in range(0 if KMINI else 4):
                for nh in range(2):
                    orow = wk.tile([128, 512], F32, tag="orow", name="orow")
                    nc.tensor.matmul(pg_dst(tt, nh), ones_row[0:1, :],
                                     bfcp_sb[0:1, 512 * nh:512 * (nh + 1)],
                                     start=False, stop=True)
                    nc.vector.tensor_tensor(
                        orow[:], pg_dst(tt, nh),
                        x2[tt][:, 512 * nh:512 * (nh + 1)], op=OP.add)
                    nc.sync.dma_start(out_rows[tt, :, 512 * nh:512 * (nh + 1)], orow[:])

    nc.compile()
    return nc


def _own_rows(c):
    rows = []
    for g in range(8):
        b, jjg = divmod(g, 4)
        base = 2048 * b + 512 * jjg + 64 * c
        rows.append(np.arange(base, base + 64))
    return np.concatenate(rows)


def _prep(inputs):
    """Host-side sharding/layout prep. Returns in_maps for the 8 cores."""
    f32 = np.float32
    bf = ml_dtypes.bfloat16
    x = np.asarray(inputs["x"], f32)
    ln1_w = np.asarray(inputs["ln1_w"], f32)
    ln1_b = np.asarray(inputs["ln1_b"], f32)
    attn_w = np.asarray(inputs["attn_w"], f32)
    attn_b = np.asarray(inputs["attn_b"], f32)
    proj_w = np.asarray(inputs["proj_w"], f32)
    proj_b = np.asarray(inputs["proj_b"], f32)
    ln2_w = np.asarray(inputs["ln2_w"], f32)
    ln2_b = np.asarray(inputs["ln2_b"], f32)
    fc_w = np.asarray(inputs["fc_w"], f32)
    fc_b = np.asarray(inputs["fc_b"], f32)
    fc_proj_w = np.asarray(inputs["fc_proj_w"], f32)
    fc_proj_b = np.asarray(inputs["fc_proj_b"], f32)

    # fold LN affine params into the following matmuls (exact linear identities)
    aw = ln1_w[:, None] * attn_w
    ab = ln1_b @ attn_w + attn_b
    fw = ln2_w[:, None] * fc_w
    fb = ln2_b @ fc_w + fc_b

    sc = 1.0 / np.sqrt(HD)
    xg = x.reshape(BT, C)                                  # global token rows
    # xT8[ch, p, 512*pt + q] = x_g[512*ch + q, 128*pt + p]
    xT8 = np.ascontiguousarray(
        xg.reshape(NCH, 512, CB, 128).transpose(0, 3, 2, 1).reshape(NCH, 128, 8 * 512)
    ).astype(bf)
    # wproj[2*nh+cbh][p, 512*cbl + j] = proj_w[128*(4*cbh+cbl) + p, 512*nh + j]
    wproj_h = np.ascontiguousarray(
        proj_w.reshape(2, 4, 128, 2, 512).transpose(3, 0, 2, 1, 4)
        .reshape(4, 128, 4 * 512)).astype(bf)
    # wfc[htg][p, 512*cb + 128*sub + j] = fw[128*cb + p, 512*htg + 128*sub + j]
    wfc_h = np.ascontiguousarray(
        fw.reshape(CB, 128, 8, 512).transpose(2, 1, 0, 3).reshape(8, 128, 8 * 512)
    ).astype(bf)
    bfc_h = np.ascontiguousarray(fb.reshape(HT, 128).T).astype(f32)
    wfcp_h = np.ascontiguousarray(fc_proj_w.reshape(HT, 128, 1024)).astype(bf)

    maskd = np.zeros((4, 128, 512), np.float32)
    for m in range(4):
        maskd[m] = (128 * m + np.arange(128)[:, None]) <= np.arange(512)[None, :]
    maskd = maskd.astype(bf)
    ident = np.eye(128, dtype=np.float32).astype(bf)

    shared = dict(
        xT8=xT8,
        wproj=wproj_h, bproj=proj_b.reshape(1, C).astype(f32),
        wfc=wfc_h, bfc=bfc_h,
        wfcp=wfcp_h, bfcp=fc_proj_b.reshape(1, C).astype(f32),
        maskd=maskd, ident=ident, onesr=np.ones((1, 128), f32),
    )

    in_maps = []
    for c in range(NCORES):
        h0 = 2 * c
        qcols = aw[:, 64 * h0:64 * h0 + 128] * sc          # [1024, 128] both heads' q
        kcols = aw[:, C + 64 * h0:C + 64 * h0 + 128]
        vcols = aw[:, 2 * C + 64 * h0:2 * C + 64 * h0 + 128]
        wqk_full = np.concatenate([qcols, kcols], axis=1).astype(bf)
        wqk_c = wqk_full.reshape(CB, 128, 256)
        bq = ab[64 * h0:64 * h0 + 128] * sc
        bk = ab[C + 64 * h0:C + 64 * h0 + 128]
        bqk0 = np.concatenate([bq, bk])
        cqk0 = wqk_full.astype(f32).sum(axis=0)

        def swap_heads(v):  # [q0 q1 k0 k1](64 each) -> [q1 q0 k1 k0]
            q, k = v[:128], v[128:]
            return np.concatenate([q[64:], q[:64], k[64:], k[:64]])

        bqk_c = np.stack([bqk0, swap_heads(bqk0)]).reshape(2, 256, 1).astype(f32)
        cqk_c = np.stack([cqk0, swap_heads(cqk0)]).reshape(2, 256, 1).astype(f32)
        wv_full = vcols.astype(bf)
        wv_c = wv_full.reshape(CB, 128, 128)
        bv_c = ab[2 * C + 64 * h0:2 * C + 64 * h0 + 128].reshape(128, 1).astype(f32)
        cv_c = wv_full.astype(f32).sum(axis=0).reshape(128, 1).astype(f32)
        xrows_c = xg[_own_rows(c)].reshape(4, 128, C).astype(f32)
        m = dict(shared)
        m.update(wqk=wqk_c, bqk=bqk_c, cqk=cqk_c, wv=wv_c, bv=bv_c, cv=cv_c,
                 xrows=xrows_c, xown=xT8[c])
        in_maps.append(m)
    return in_maps


def kernel(**inputs) -> np.ndarray:
    if "nc" not in _cache:
        _cache["nc"] = build()
    nc = _cache["nc"]
    in_maps = _prep(inputs)
    res = bass_utils.run_bass_kernel_spmd(nc, in_maps, core_ids=list(range(NCORES)))
    out = np.empty((BT, C), np.float32)
    for c in range(NCORES):
        out[_own_rows(c)] = res.results[c]["out_rows"].reshape(RPC, C)
    return out.reshape(B, T, C)


# revision 22
# speedup vs baseline: 1.1033x; 1.1033x over previous
"""Trainium2 Bass kernel for a pre-LN transformer block (B=2, T=2048, C=1024, H=16).

Strategy (8 NeuronCores, SPMD), v3 pipelined:
  - Tensor-parallel over heads for attention: core c owns heads {2c, 2c+1}.
  - LN1 stats distributed: core c computes mean/rsqrt rows for token chunk c
    only, then one 4KB AllGather shares all 8 chunks' stats; the LN1 affine is
    folded post-matmul into qkv as before.
  - jj-major software pipeline: for each q-chunk jj (512 tokens per batch),
    emit qkv for chunks (b0,jj),(b1,jj), then causal attention for q-chunk jj
    (all keys <= jj are ready), then a small per-jj AllToAll. Keeps the PE
    dense (HAM stays warm) and overlaps attention's Scalar-bound exp with the
    next chunk's matmuls.
  - Row ownership is 64-token interleaved: core c owns rows [64c, 64c+64) of
    every (batch, jj) 512-chunk, so each per-jj A2A feeds every core and only
    the last small A2A is exposed.
  - exp batched 2 key-tiles per ACT ([128,1024] PSUM read) to amortize the
    352-cycle ACT overhead; causal masks multiplied on VectorE (bf16 2x).
  - Row-parallel proj + LN2 + MLP on the core's own 512 rows.

PSUM budget (8 banks): tag "w2" = 3 x [128,1024] f32 (6 banks) +
tag "po" = 2 x [65,512] f32 (2 banks).
"""

import os
import numpy as np
import ml_dtypes

from concourse import bass, bacc, tile, mybir, bass_utils

BF16 = mybir.dt.bfloat16
F32 = mybir.dt.float32
F32R = mybir.dt.float32r
AX = mybir.AxisListType
OP = mybir.AluOpType
AF = mybir.ActivationFunctionType

B, T, C, H, HD = 2, 2048, 1024, 16, 64
NCORES = 8
BT = B * T                  # 4096 global tokens
RPC = BT // NCORES          # 512 rows per core
NCH = BT // 512             # 8 token chunks of 512
CB = C // 128               # 8 contraction blocks
HT = 4 * C // 128           # 32 hidden tiles
EPS = 1e-5

_cache = {}


def _fr(ap):
    return ap.bitcast(F32R)


def build():
    nc = bacc.Bacc("TRN2", target_bir_lowering=False, debug=False, num_devices=NCORES)

    def din(name, shape, dt=BF16):
        return nc.dram_tensor(name, list(shape), dt, kind="ExternalInput").ap()

    xT8 = din("xT8", [NCH, 128, 8 * 512])                 # x transposed, chunked
    xown = din("xown", [128, 8 * 512])                    # own chunk (stats)
    xrows = din("xrows", [4, 128, C], F32)                # own residual rows
    wqk = din("wqk", [CB, 128, 256])                      # [q_h0|q_h1|k_h0|k_h1]
    bqk = din("bqk", [2, 256, 1], F32)
    cqk = din("cqk", [2, 256, 1], F32)   # [b0-order, b1-swapped]
    wv = din("wv", [CB, 128, 128])                        # [v_h0|v_h1]
    bv = din("bv", [128, 1], F32)
    cv = din("cv", [128, 1], F32)
    wproj = din("wproj", [4, 128, 4 * 512])
    bproj = din("bproj", [1, C], F32R)
    wfc = din("wfc", [8, 128, 8 * 512])                   # htg -> [cb | 4 ht cols]
    bfc = din("bfc", [128, HT], F32)
    wfcp = din("wfcp", [HT, 128, 1024])
    bfcp = din("bfcp", [1, C], F32R)
    maskd = din("maskd", [4, 128, 512])                   # 0/1 causal diag masks
    ident = din("ident", [128, 128])
    onesr = din("onesr", [1, 128], F32R)
    out_rows = nc.dram_tensor("out_rows", [4, 128, C], F32, kind="ExternalOutput").ap()

    with tile.TileContext(nc) as tc:
        with tc.tile_pool(name="persist", bufs=1) as pp, \
             tc.tile_pool(name="work", bufs=2) as wk, \
             tc.tile_pool(name="psum", bufs=1, space="PSUM") as ps, \
             tc.tile_pool(name="dram", bufs=1, space="DRAM") as dram:

            # ---------- constants / small persistent tiles ----------
            ones_bf = pp.tile([128, 1], BF16, tag="ones_bf")
            nc.vector.memset(ones_bf[:], 1.0 / C)
            ones_row = pp.tile([1, 128], F32R, tag="ones_row")
            nc.sync.dma_start(ones_row[:], onesr[:])
            eps1 = pp.tile([1, 1], F32, tag="eps1")
            nc.vector.memset(eps1[:], EPS)
            cci = dram.tile([1, 1024], F32, tag="cci")
            cco = dram.tile([8, 1024], F32, tag="cco")
            # ===== Stats pass: own chunk only, then AllGather =====
            sx = wk.tile([128, 8 * 512], BF16, tag="xt", bufs=2, name="sx")
            nc.sync.dma_start(sx[:], xown[:])
            st1 = ps.tile([1, 512], F32, tag="po", bufs=2)
            st2 = ps.tile([1, 512], F32, tag="po", bufs=2)
            for pt in range(CB):
                nc.tensor.matmul(st1[:], ones_bf[:], sx[:, 512 * pt:512 * (pt + 1)],
                                 start=(pt == 0), stop=(pt == CB - 1))
            for h in range(4):
                sqa = wk.tile([128, 2 * 512], BF16, tag="sq", bufs=1, name="sqa")
                nc.vector.tensor_tensor(
                    sqa[:], sx[:, 1024 * h:1024 * (h + 1)],
                    sx[:, 1024 * h:1024 * (h + 1)], op=OP.mult)
                for pp_i in range(2):
                    pt = 2 * h + pp_i
                    nc.tensor.matmul(st2[:], ones_bf[:],
                                     sqa[:, 512 * pp_i:512 * (pp_i + 1)],
                                     start=(pt == 0), stop=(pt == CB - 1))
            mu2 = wk.tile([1, 512], F32, tag="arow", bufs=2, name="mu2")
            nc.scalar.activation(mu2[:], st1[:], AF.Square)
            var = wk.tile([1, 512], F32, tag="arow", bufs=2, name="var")
            nc.vector.tensor_tensor(var[:], st2[:], mu2[:], op=OP.subtract)
            rs_r = wk.tile([1, 512], F32, tag="arow", bufs=2, name="rs_r")
            nc.scalar.activation(rs_r[:], var[:], AF.Abs_reciprocal_sqrt, bias=eps1[:])
            am_r = wk.tile([1, 512], F32, tag="arow", bufs=2, name="am_r")
            nc.vector.tensor_tensor(am_r[:], st1[:], rs_r[:], op=OP.mult)
            nc.sync.dma_start(cci[0:1, 0:512], rs_r[:])
            nc.sync.dma_start(cci[0:1, 512:1024], am_r[:])
            nc.gpsimd.collective_compute(
                "AllGather", OP.bypass,
                ins=[cci.opt()], outs=[cco.opt()],
                replica_groups=[list(range(NCORES))],
            )
            eps128 = pp.tile([128, 1], F32, tag="eps128")
            nc.vector.memset(eps128[:], EPS)
            idn = pp.tile([128, 128], BF16, tag="idn")
            nc.sync.dma_start(idn[:], ident[:])
            msk = [pp.tile([128, 512], BF16, tag=f"msk{m}", name=f"msk{m}") for m in range(4)]
            for m in range(4):
                nc.sync.dma_start(msk[m][:], maskd[m])
            bq_sb = [pp.tile([128, 1], F32, tag=f"bq{v_}", name=f"bq{v_}") for v_ in range(2)]
            bk_sb = [pp.tile([128, 1], F32, tag=f"bk{v_}", name=f"bk{v_}") for v_ in range(2)]
            cq_sb = [pp.tile([128, 1], F32, tag=f"cq{v_}", name=f"cq{v_}") for v_ in range(2)]
            ck_sb = [pp.tile([128, 1], F32, tag=f"ck{v_}", name=f"ck{v_}") for v_ in range(2)]
            for v_ in range(2):
                nc.sync.dma_start(bq_sb[v_][:], bqk[v_, 0:128, :])
                nc.sync.dma_start(bk_sb[v_][:], bqk[v_, 128:256, :])
                nc.sync.dma_start(cq_sb[v_][:], cqk[v_, 0:128, :])
                nc.sync.dma_start(ck_sb[v_][:], cqk[v_, 128:256, :])
            bv_sb = pp.tile([128, 1], F32, tag="bv_sb")
            nc.sync.dma_start(bv_sb[:], bv[:])
            cv_sb = pp.tile([128, 1], F32, tag="cv_sb")
            nc.sync.dma_start(cv_sb[:], cv[:])
            bproj_sb = pp.tile([1, C], F32R, tag="bproj_sb")
            nc.sync.dma_start(bproj_sb[:], bproj[:])
            bfc_sb = pp.tile([128, HT], F32, tag="bfc_sb")
            nc.sync.dma_start(bfc_sb[:], bfc[:])
            bfcp_sb = pp.tile([1, C], F32R, tag="bfcp_sb")
            nc.sync.dma_start(bfcp_sb[:], bfcp[:])
            wqk_sb = [pp.tile([128, 256], BF16, tag=f"wqk{cb}", name=f"wqk{cb}") for cb in range(CB)]
            wv_sb = [pp.tile([128, 128], BF16, tag=f"wv{cb}", name=f"wvsb{cb}") for cb in range(CB)]
            for cb in range(CB):
                nc.sync.dma_start(wqk_sb[cb][:], wqk[cb])
                nc.sync.dma_start(wv_sb[cb][:], wv[cb])

            # persistent activation tensors, head-major:
            qt = [pp.tile([128, T], BF16, tag=f"qt{X}", name=f"qt{X}") for X in range(2)]
            kt_ = [pp.tile([128, T], BF16, tag=f"kt{X}", name=f"ktt{X}") for X in range(2)]
            # v transposed, rows = key position within its 128-tile:
            # v1big[b][:, 130*kti + 65*h + d], col 65*h+64 = ones (denominator)
            v1big = [pp.tile([128, 16 * 130], BF16, tag=f"v1b{b}", name=f"v1b{b}")
                     for b in range(B)]
            for b in range(B):
                vr = v1big[b].rearrange("p (k h d) -> p k h d", k=16, h=2)
                nc.vector.memset(vr[:, :, :, 64:65], 1.0)

            # residual rows + A2A-assembled yT
            x2 = [pp.tile([128, C], F32, tag=f"x2{tt}", name=f"x2{tt}") for tt in range(4)]
            yTbig = pp.tile([128, 8 * 512], BF16, tag="yTbig")
            wpj = [pp.tile([128, 4 * 512], BF16, tag=f"wpj{q}", name=f"wpj{q}")
                   for q in range(4)]
            ln2Tbig = pp.tile([128, 8 * 512], BF16, tag="ln2Tbig")
            ghT = [pp.tile([128, 512], BF16, tag=f"ghT{ht}", name=f"ghT{ht}") for ht in range(HT)]

            # collective DRAM buffers
            ib = [dram.tile([8, 128, 128], BF16, tag=f"ib{j}", name=f"ib{j}") for j in range(4)]
            ob = [dram.tile([8, 128, 128], BF16, tag=f"ob{j}", name=f"ob{j}") for j in range(4)]
            ib3x = [dram.tile([8, 64, 128], BF16, tag=f"ib3x{xx}", name=f"ib3x{xx}") for xx in range(2)]
            ob3x = [dram.tile([8, 64, 128], BF16, tag=f"ob3x{xx}", name=f"ob3x{xx}") for xx in range(2)]

            # ---------- helpers ----------
            KMINI = int(os.environ.get("KMINI", "0"))
            def emit_chunk(jj, b):
                """LN1 (folded post-matmul) + qkv (transposed) for chunk (b, jj)."""
                ch = 4 * b + jj
                xta = wk.tile([128, 8 * 512], BF16, tag="xt", bufs=2, name="xta")
                nc.sync.dma_start(xta[:], xT8[ch])
                xt = [xta[:, 512 * pt:512 * (pt + 1)] for pt in range(CB)]
                strow = wk.tile([1, 1024], F32, tag="strow", bufs=2, name="strow")
                nc.scalar.dma_start(strow[:], cco[ch:ch + 1, :])
                bc_rs = wk.tile([128, 512], F32, tag="bc", bufs=2, name="bc_rs")
                bc_a = wk.tile([128, 512], F32, tag="bc", bufs=2, name="bc_a")
                nc.gpsimd.partition_broadcast(bc_rs[:], strow[0:1, 0:512])
                nc.gpsimd.partition_broadcast(bc_a[:], strow[0:1, 512:1024])

                def qk_mms(psum, base):
                    for pt in range(CB):
                        if b == 0:
                            nc.tensor.matmul(psum, wqk_sb[pt][:, base:base + 128],
                                             xt[pt], start=(pt == 0),
                                             stop=(pt == CB - 1))
                        else:
                            nc.tensor.matmul(psum[0:64, :],
                                             wqk_sb[pt][:, base + 64:base + 128],
                                             xt[pt], start=(pt == 0),
                                             stop=(pt == CB - 1))
                            nc.tensor.matmul(psum[64:128, :],
                                             wqk_sb[pt][:, base:base + 64],
                                             xt[pt], start=(pt == 0),
                                             stop=(pt == CB - 1),
                                             tile_position=(0, 64))

                def fold_qk(dst_pair, gp, cs_ap, b_ap):
                    # dst = rs*G - (bc_a*cs - b), written per partition half
                    m = wk.tile([128, 512], BF16, tag="foldm", bufs=2, name="m")
                    nc.vector.tensor_scalar(m[:], bc_a[:], cs_ap, b_ap,
                                            op0=OP.mult, op1=OP.subtract)
                    p1 = wk.tile([128, 512], BF16, tag="foldp", bufs=2, name="p1")
                    nc.vector.tensor_tensor(p1[:], gp, bc_rs[:], op=OP.mult)
                    js = slice(512 * jj, 512 * (jj + 1))
                    nc.vector.tensor_tensor(dst_pair[0][0:64, js], p1[0:64, :],
                                            m[0:64, :], op=OP.subtract)
                    nc.vector.tensor_tensor(dst_pair[1][64:128, js], p1[64:128, :],
                                            m[64:128, :], op=OP.subtract)

                dq = (qt[0], qt[1]) if b == 0 else (qt[1], qt[0])
                dk = (kt_[0], kt_[1]) if b == 0 else (kt_[1], kt_[0])
                qk2 = ps.tile([128, 1024], F32, tag="w2", bufs=3)
                qk_mms(qk2[:, 0:512], 0)
                qk_mms(qk2[:, 512:1024], 128)
                fold_qk(dq, qk2[:, 0:512], cq_sb[b][:], bq_sb[b][:])
                fold_qk(dk, qk2[:, 512:1024], ck_sb[b][:], bk_sb[b][:])
                # v (transposed) then per-128 transpose into row-layout v1big
                pv2 = ps.tile([128, 1024], F32, tag="w2", bufs=3)
                pv = pv2[:, 0:512]
                for pt in range(CB):
                    nc.tensor.matmul(pv, wv_sb[pt][:], xt[pt],
                                     start=(pt == 0), stop=(pt == CB - 1))
                vts = wk.tile([128, 512], BF16, tag="vts", bufs=2)
                m = wk.tile([128, 512], BF16, tag="foldm", bufs=2, name="m")
                nc.vector.tensor_scalar(m[:], bc_a[:], cv_sb[:], bv_sb[:],
                                        op0=OP.mult, op1=OP.subtract)
                p1 = wk.tile([128, 512], BF16, tag="foldp", bufs=2, name="p1")
                nc.vector.tensor_tensor(p1[:], pv, bc_rs[:], op=OP.mult)
                nc.vector.tensor_tensor(vts[:], p1[:], m[:], op=OP.subtract)
                ptr4 = ps.tile([128, 512], BF16, tag="po", bufs=2)
                for t in range(4):
                    nc.tensor.transpose(ptr4[:, 128 * t:128 * (t + 1)],
                                        vts[:, 128 * t:128 * (t + 1)], idn[:])
                dst = v1big[b].rearrange("p (k h d) -> p k h d", k=16, h=2)
                nc.vector.tensor_copy(
                    dst[:, 4 * jj:4 * jj + 4, :, 0:64],
                    ptr4[:].rearrange("p (k h d) -> p k h d", k=4, h=2))

            def emit_attn(jj):
                """Causal attention for q-chunk jj (both heads X, both batches)."""
                nkt = 4 * jj + 4
                for X in range(2):
                    for u in range(2):
                        b = u if X == 0 else 1 - u
                        po_t = ps.tile([65, 512], F32, tag="po", bufs=2)
                        for mb in range(nkt // 2):
                            s2 = ps.tile([128, 1024], F32, tag="w2", bufs=3)
                            for i in range(2):
                                kti = 2 * mb + i
                                nc.tensor.matmul(
                                    s2[:, 512 * i:512 * (i + 1)],
                                    kt_[X][64 * u:64 * (u + 1), 128 * kti:128 * (kti + 1)],
                                    qt[X][64 * u:64 * (u + 1), 512 * jj:512 * (jj + 1)],
                                    start=True, stop=True)
                            pt_sb = wk.tile([128, 1024], BF16, tag="ptb", bufs=3, name="pt_sb")
                            nc.scalar.activation(pt_sb[:], s2[:], AF.Exp)
                            for i in range(2):
                                kti = 2 * mb + i
                                mrel = kti - 4 * jj
                                sl = pt_sb[:, 512 * i:512 * (i + 1)]
                                if mrel >= 0:
                                    nc.vector.tensor_tensor(sl, sl, msk[mrel][:], op=OP.mult)
                                nc.tensor.matmul(
                                    po_t[:],
                                    v1big[b][:, 130 * kti + 65 * X:130 * kti + 65 * X + 65],
                                    sl, start=(kti == 0), stop=(kti == nkt - 1))
                        dcp = wk.tile([1, 512], F32, tag="dcp", bufs=1, name="dcp")
                        nc.vector.tensor_copy(dcp[:], po_t[64:65, :])
                        recip = wk.tile([1, 512], F32, tag="recip", bufs=1)
                        nc.vector.reciprocal_approx_fast(recip[:], dcp[:])
                        bcp = wk.tile([64, 512], F32, tag="bcb", bufs=1, name="bcp")
                        nc.gpsimd.partition_broadcast(bcp[:], recip[:])
                        yt = wk.tile([64, 512], BF16, tag="yt", bufs=2)
                        nc.vector.tensor_tensor(yt[:], po_t[0:64, :], bcp[:], op=OP.mult)
                        if jj == 3:
                            nc.sync.dma_start(
                                ib3x[X][:, :, 64 * b:64 * b + 64]
                                .rearrange("c p t -> p c t"),
                                yt[:].rearrange("p (c t) -> p c t", c=8))
                        else:
                            nc.sync.dma_start(
                                ib[jj][:, 64 * X:64 * X + 64, 64 * b:64 * b + 64]
                                .rearrange("c p t -> p c t"),
                                yt[:].rearrange("p (c t) -> p c t", c=8))
                    if jj == 3:
                        nc.gpsimd.collective_compute(
                            "AllToAll", OP.bypass,
                            ins=[ib3x[X].opt()], outs=[ob3x[X].opt()],
                            replica_groups=[list(range(NCORES))],
                        )
                if jj != 3:
                    nc.gpsimd.collective_compute(
                        "AllToAll", OP.bypass,
                        ins=[ib[jj].opt()], outs=[ob[jj].opt()],
                        replica_groups=[list(range(NCORES))],
                    )

            def emit_assembly(jj):
                """Scatter A2A output ob[jj] into yTbig columns."""
                dst = yTbig.rearrange("p (s b g t) -> p s b g t", s=8, b=2, g=4)
                for b in range(2):
                    if jj == 3:
                        for X in range(2):
                            nc.sync.dma_start(
                                dst[64 * X:64 * X + 64, :, b, jj, :],
                                ob3x[X][:, :, 64 * b:64 * b + 64]
                                .rearrange("s p t -> p s t"))
                    else:
                        nc.sync.dma_start(
                            dst[:, :, b, jj, :],
                            ob[jj][:, :, 64 * b:64 * b + 64].rearrange("s p t -> p s t"))

            def emit_proj(tt):
                """proj + residual for token tile tt (128 own rows)."""
                pps = ps.tile([128, 1024], F32, tag="w2", bufs=3)
                for nh in range(2):
                    dst = pps[:, 512 * nh:512 * (nh + 1)]
                    for cbh in range(2):
                        q = 2 * nh + cbh
                        for cbl in range(4):
                            cb = 4 * cbh + cbl
                            nc.tensor.matmul(
                                dst,
                                yTbig[:, 512 * cb + 128 * tt:512 * cb + 128 * (tt + 1)],
                                wpj[q][:, 512 * cbl:512 * (cbl + 1)],
                                start=(cb == 0), stop=False)
                    nc.tensor.matmul(dst, ones_row[0:1, :],
                                     bproj_sb[0:1, 512 * nh:512 * (nh + 1)],
                                     start=False, stop=True)
                nc.vector.tensor_tensor(x2[tt][:], pps[:], x2[tt][:], op=OP.add)

            def emit_ln2(tt):
                """LN2 + transpose into ln2Tbig for token tile tt."""
                s1 = wk.tile([128, 1], F32, tag="e_s1")
                nc.vector.reduce_sum(s1[:], x2[tt][:], axis=AX.X)
                nmu = wk.tile([128, 1], F32, tag="e_nmu")
                nc.vector.tensor_scalar(nmu[:], s1[:], -1.0 / C, None, op0=OP.mult)
                sqs = wk.tile([128, C], F32, tag="sq", bufs=1, name="sqs")
                s2_ = wk.tile([128, 1], F32, tag="e_s2")
                nc.scalar.activation(sqs[:], x2[tt][:], AF.Square, accum_out=s2_[:])
                m2 = wk.tile([128, 1], F32, tag="e_m2")
                nc.vector.tensor_tensor(m2[:], nmu[:], nmu[:], op=OP.mult)
                var_ = wk.tile([128, 1], F32, tag="e_var")
                nc.vector.tensor_scalar(var_[:], s2_[:], 1.0 / C, None, op0=OP.mult)
                nc.vector.tensor_tensor(var_[:], var_[:], m2[:], op=OP.subtract)
                sd = wk.tile([128, 1], F32, tag="e_sd")
                nc.scalar.activation(sd[:], var_[:], AF.Sqrt, bias=eps128[:])
                rs2 = wk.tile([128, 1], F32, tag="e_rs2")
                nc.vector.reciprocal(rs2[:], sd[:])
                na = wk.tile([128, 1], F32, tag="e_na")
                nc.vector.tensor_tensor(na[:], nmu[:], rs2[:], op=OP.mult)
                lr = wk.tile([128, C], BF16, tag="e_lr", bufs=1)
                nc.scalar.activation(lr[:], x2[tt][:], AF.Identity,
                                     bias=na[:], scale=rs2[:])
                ldst = ln2Tbig.rearrange("p (cb t) -> p cb t", cb=8)
                for hh in range(2):
                    ptr4 = ps.tile([128, 512], BF16, tag="po", bufs=2)
                    for t in range(4):
                        cb = 4 * hh + t
                        nc.tensor.transpose(ptr4[:, 128 * t:128 * (t + 1)],
                                            lr[:, 128 * cb:128 * (cb + 1)], idn[:])
                    nc.vector.tensor_copy(
                        ldst[:, 4 * hh:4 * hh + 4, 128 * tt:128 * (tt + 1)],
                        ptr4[:].rearrange("p (k t) -> p k t", k=4))

            # =================== main pipeline ===================
            if KMINI:
                emit_chunk(0, 0)
                emit_chunk(0, 1)
                emit_attn(0)
                emit_assembly(0)
            else:
                for jj in range(4):
                    emit_chunk(jj, 0)
                    emit_chunk(jj, 1)
                    if jj == 2:
                        for tt in range(4):
                            nc.sync.dma_start(x2[tt][:], xrows[tt])
                        for q in range(4):
                            nc.sync.dma_start(wpj[q][:], wproj[q])
                    if jj == 3:
                        emit_proj(0)
                        emit_ln2(0)
                        emit_proj(2)
                        emit_ln2(2)
                    emit_attn(jj)
                    if jj >= 1:
                        emit_assembly(jj - 1)
                emit_assembly(3)
                emit_proj(1)
                emit_ln2(1)
                emit_proj(3)
                emit_ln2(3)

            # =========== MLP: fc + gelu ===========
            for htg in range(0 if KMINI else 8):
                wq = []
                for qq_ in range(4):
                    wqt = wk.tile([128, 2 * 512], BF16, tag="wfc", bufs=6, name="wqt")
                    nc.sync.dma_start(wqt[:], wfc[htg, :, 1024 * qq_:1024 * (qq_ + 1)])
                    wq.append(wqt)
                for sub in range(4):
                    ht = 4 * htg + sub
                    ph = ps.tile([128, 512], F32, tag="po", bufs=2)
                    for cb in range(CB):
                        wt = wq[cb // 2]
                        cbl = cb % 2
                        nc.tensor.matmul(
                            ph[:], wt[:, 512 * cbl + 128 * sub:512 * cbl + 128 * (sub + 1)],
                            ln2Tbig[:, 512 * cb:512 * (cb + 1)],
                            start=(cb == 0), stop=(cb == CB - 1))
                    nc.scalar.activation(ghT[ht][:], ph[:], AF.Gelu,
                                         bias=bfc_sb[:, ht:ht + 1])

            # =========== MLP: fc_proj + residual -> output rows ===========
            pg = {}
            for tt in range(0 if KMINI else 3):
                pg[tt] = ps.tile([128, 1024], F32, tag="w2", bufs=3, name=f"pg{tt}")
            if not KMINI:
                pg3a = ps.tile([128, 512], F32, tag="po", bufs=2, name="pg3a")
                pg3b = ps.tile([128, 512], F32, tag="po", bufs=2, name="pg3b")

            def pg_dst(tt, nh):
                if tt < 3:
                    return pg[tt][:, 512 * nh:512 * (nh + 1)]
                return pg3a[:] if nh == 0 else pg3b[:]

            for ht in range(0 if KMINI else HT):
                w = wk.tile([128, 1024], BF16, tag="wfcp", bufs=4, name="wfcp")
                nc.gpsimd.dma_start(w[:], wfcp[ht])
                for tt in range(4):
                    for nh in range(2):
                        nc.tensor.matmul(pg_dst(tt, nh),
                                         ghT[ht][:, 128 * tt:128 * (tt + 1)],
                                         w[:, 512 * nh:512 * (nh + 1)],
                                         start=(ht == 0), stop=False)
            for tt in range(0 if KMINI else 4):
                for nh in range(2):
                    orow = wk.tile([128, 512], F32, tag="orow", name="orow")
                    nc.tensor.matmul(pg_dst(tt, nh), ones_row[0:1, :],
                                     bfcp_sb[0:1, 512 * nh:512 * (nh + 1)],
                                     start=False, stop=True)
                    nc.vector.tensor_tensor(
                        orow[:], pg_dst(tt, nh),
                        x2[tt][:, 512 * nh:512 * (nh + 1)], op=OP.add)
                    nc.sync.dma_start(out_rows[tt, :, 512 * nh:512 * (nh + 1)], orow[:])

    nc.compile()
    return nc


def _own_rows(c):
    rows = []
    for g in range(8):
        b, jjg = divmod(g, 4)
        base = 2048 * b + 512 * jjg + 64 * c
        rows.append(np.arange(base, base + 64))
    return np.concatenate(rows)


def _prep(inputs):
    """Host-side sharding/layout prep. Returns in_maps for the 8 cores."""
    f32 = np.float32
    bf = ml_dtypes.bfloat16
    x = np.asarray(inputs["x"], f32)
    ln1_w = np.asarray(inputs["ln1_w"], f32)
    ln1_b = np.asarray(inputs["ln1_b"], f32)
    attn_w = np.asarray(inputs["attn_w"], f32)
    attn_b = np.asarray(inputs["attn_b"], f32)
    proj_w = np.asarray(inputs["proj_w"], f32)
    proj_b = np.asarray(inputs["proj_b"], f32)
    ln2_w = np.asarray(inputs["ln2_w"], f32)
    ln2_b = np.asarray(inputs["ln2_b"], f32)
    fc_w = np.asarray(inputs["fc_w"], f32)
    fc_b = np.asarray(inputs["fc_b"], f32)
    fc_proj_w = np.asarray(inputs["fc_proj_w"], f32)
    fc_proj_b = np.asarray(inputs["fc_proj_b"], f32)

    # fold LN affine params into the following matmuls (exact linear identities)
    aw = ln1_w[:, None] * attn_w
    ab = ln1_b @ attn_w + attn_b
    fw = ln2_w[:, None] * fc_w
    fb = ln2_b @ fc_w + fc_b

    sc = 1.0 / np.sqrt(HD)
    xg = x.reshape(BT, C)                                  # global token rows
    # xT8[ch, p, 512*pt + q] = x_g[512*ch + q, 128*pt + p]
    xT8 = np.ascontiguousarray(
        xg.reshape(NCH, 512, CB, 128).transpose(0, 3, 2, 1).reshape(NCH, 128, 8 * 512)
    ).astype(bf)
    # wproj[2*nh+cbh][p, 512*cbl + j] = proj_w[128*(4*cbh+cbl) + p, 512*nh + j]
    wproj_h = np.ascontiguousarray(
        proj_w.reshape(2, 4, 128, 2, 512).transpose(3, 0, 2, 1, 4)
        .reshape(4, 128, 4 * 512)).astype(bf)
    # wfc[htg][p, 512*cb + 128*sub + j] = fw[128*cb + p, 512*htg + 128*sub + j]
    wfc_h = np.ascontiguousarray(
        fw.reshape(CB, 128, 8, 512).transpose(2, 1, 0, 3).reshape(8, 128, 8 * 512)
    ).astype(bf)
    bfc_h = np.ascontiguousarray(fb.reshape(HT, 128).T).astype(f32)
    wfcp_h = np.ascontiguousarray(fc_proj_w.reshape(HT, 128, 1024)).astype(bf)

    maskd = np.zeros((4, 128, 512), np.float32)
    for m in range(4):
        maskd[m] = (128 * m + np.arange(128)[:, None]) <= np.arange(512)[None, :]
    maskd = maskd.astype(bf)
    ident = np.eye(128, dtype=np.float32).astype(bf)

    shared = dict(
        xT8=xT8,
        wproj=wproj_h, bproj=proj_b.reshape(1, C).astype(f32),
        wfc=wfc_h, bfc=bfc_h,
        wfcp=wfcp_h, bfcp=fc_proj_b.reshape(1, C).astype(f32),
        maskd=maskd, ident=ident, onesr=np.ones((1, 128), f32),
    )

    in_maps = []
    for c in range(NCORES):
        h0 = 2 * c
        qcols = aw[:, 64 * h0:64 * h0 + 128] * sc          # [1024, 128] both heads' q
        kcols = aw[:, C + 64 * h0:C + 64 * h0 + 128]
        vcols = aw[:, 2 * C + 64 * h0:2 * C + 64 * h0 + 128]
        wqk_full = np.concatenate([qcols, kcols], axis=1).astype(bf)
        wqk_c = wqk_full.reshape(CB, 128, 256)
        bq = ab[64 * h0:64 * h0 + 128] * sc
        bk = ab[C + 64 * h0:C + 64 * h0 + 128]
        bqk0 = np.concatenate([bq, bk])
        cqk0 = wqk_full.astype(f32).sum(axis=0)

        def swap_heads(v):  # [q0 q1 k0 k1](64 each) -> [q1 q0 k1 k0]
            q, k = v[:128], v[128:]
            return np.concatenate([q[64:], q[:64], k[64:], k[:64]])

        bqk_c = np.stack([bqk0, swap_heads(bqk0)]).reshape(2, 256, 1).astype(f32)
        cqk_c = np.stack([cqk0, swap_heads(cqk0)]).reshape(2, 256, 1).astype(f32)
        wv_full = vcols.astype(bf)
        wv_c = wv_full.reshape(CB, 128, 128)
        bv_c = ab[2 * C + 64 * h0:2 * C + 64 * h0 + 128].reshape(128, 1).astype(f32)
        cv_c = wv_full.astype(f32).sum(axis=0).reshape(128, 1).astype(f32)
        xrows_c = xg[_own_rows(c)].reshape(4, 128, C).astype(f32)
        m = dict(shared)
        m.update(wqk=wqk_c, bqk=bqk_c, cqk=cqk_c, wv=wv_c, bv=bv_c, cv=cv_c,
                 xrows=xrows_c, xown=xT8[c])
        in_maps.append(m)
    return in_maps


def kernel(**inputs) -> np.ndarray:
    if "nc" not in _cache:
        _cache["nc"] = build()
    nc = _cache["nc"]
    in_maps = _prep(inputs)
    res = bass_utils.run_bass_kernel_spmd(nc, in_maps, core_ids=list(range(NCORES)))
    out = np.empty((BT, C), np.float32)
    for c in range(NCORES):
        out[_own_rows(c)] = res.results[c]["out_rows"].reshape(RPC, C)
    return out.reshape(B, T, C)
